# revision 99
# baseline (speedup 1.0000x reference)
"""Trainium2 Bass kernel for nn_Attention_9612136808713.

Transformer-XL style attention (rel-shift pos bias, causal, 16 heads),
b=2, n=2048, dim=1024. Sharded over 8 NeuronCores: data-parallel on
batch (2) x tensor-parallel on heads (4 groups of 4 heads). Wq/Wkv
column-split per head group; Wo row-split with the partial-sum
reduction done on the host during unsharding.

Design notes (v3, factored-exp + interleaved schedule):
  scores S = qk*scale + shift(q.p^T*scale).  exp(S) = exp(qk) *
  E_shifted with E = exp(q.p^T*scale) materialised through the DRAM
  rel-shift roundtrip; no identity-matmul re-add, and the causal mask
  comes free (masked lanes read the shifted layout's spill region and
  are zeroed by one affine_select per row-block).  The softmax
  denominator rides along as an all-ones 65th column of v.  attn@v
  runs in the flipped [i-part, d-free] layout; one [128,128]
  transpose per (pair, row-block) restores [d-part, i-free] for the
  output projection.  All PSUM flows through three shared pools so no
  phase barriers exist; phases are emitted interleaved (B(p0) between
  the p0/p1 projection GEMMs, out-projection blocks inside the C(p1)
  loop) to keep every engine fed.

Self-contained: only needs numpy + the concourse/bass toolchain that is
installed in the environment.
"""

import contextlib
import json

import numpy as np

import concourse.bass as bass
import concourse.mybir as mybir
import concourse.tile as tile
from concourse.bass_utils import run_bass_kernel_spmd

F32 = mybir.dt.float32
FP16 = mybir.dt.float16  # 10-bit mantissa; all values here are bounded
ALU = mybir.AluOpType

N = 2048
DIM = 1024
HEADS = 16
D = 64          # head dim
HPC = 4         # heads per core
PAIRS = 2       # head pairs per core
CH = 512        # free-dim chunk (one PSUM bank of fp32)
NB = N // 128   # 16 row blocks
KC = DIM // 128  # 8 contraction chunks
SCALE = D ** -0.5

# engine-routing / buffering knobs
SPP_BUFS = 3
TPP_BUFS = 2
PT_BUFS = 4
POS_BUFS = 3
UB_BUFS = 2


# --------------------------------------------------------------------------
# Wait-splitting post-pass: this container's walrus build accepts only ONE
# sync-wait command per instruction, while Tile attaches several. Splitting
# an AND-wait into single-wait NoOps on the same engine immediately before
# the instruction is semantically equivalent (sem-ge waits are monotonic).
# --------------------------------------------------------------------------

def _split_waits_json_bytes(raw: bytes) -> bytes:
    d = json.loads(raw)
    counter = [0]

    def fix_block(b):
        out = []
        for inst in b.get("instructions", []):
            si = inst.get("sync_info")
            waits = (si or {}).get("on_wait") or []
            if len(waits) > 1:
                eng = inst.get("engine")
                for w in waits[:-1]:
                    counter[0] += 1
                    out.append(
                        {
                            "engine": eng,
                            "ins": [],
                            "outs": [],
                            "name": f"WSPLIT-{counter[0]}",
                            "opcode": "NoOp",
                            "sync_info": {"on_update": [], "on_wait": [w]},
                        }
                    )
                si["on_wait"] = [waits[-1]]
            out.append(inst)
        b["instructions"] = out

    for f in d.get("functions", []):
        for b in f.get("blocks", []):
            fix_block(b)
    return json.dumps(d).encode()


def _patch_bass(nc):
    orig = nc.to_json_bytes

    def patched():
        return _split_waits_json_bytes(orig())

    nc.to_json_bytes = patched
    return nc


def build_nc(debug=False):
    nc = bass.Bass()

    xT = nc.dram_tensor("xT", [DIM, N], FP16, kind="ExternalInput")
    posT = nc.dram_tensor("posT", [DIM, N], FP16, kind="ExternalInput")
    wq = nc.dram_tensor("wq", [DIM, 256], FP16, kind="ExternalInput")
    wk = nc.dram_tensor("wk", [DIM, 256], FP16, kind="ExternalInput")
    wv = nc.dram_tensor("wv", [DIM, 256], FP16, kind="ExternalInput")
    wpd = nc.dram_tensor("wpd", [DIM, 128], FP16, kind="ExternalInput")
    wo = nc.dram_tensor("wo", [256, DIM], FP16, kind="ExternalInput")
    bq = nc.dram_tensor("bq", [256, 1], F32, kind="ExternalInput")
    bks = nc.dram_tensor("bks", [256, 1], F32, kind="ExternalInput")  # SCALE*bk
    bpd = nc.dram_tensor("bpd", [128, 1], F32, kind="ExternalInput")  # SCALE*bp dup
    ident = nc.dram_tensor("ident", [128, 128], FP16, kind="ExternalInput")
    out = nc.dram_tensor("out", [N, DIM], FP16, kind="ExternalOutput")

    # rel-shift scratch: E = exp(scale * q.p^T), one [N, N] fp16 slab per head
    ebkind = "ExternalOutput" if debug else "Internal"
    EB = [nc.dram_tensor(f"EB{p}", [2, N * N], FP16, kind=ebkind) for p in range(PAIRS)]
    if debug:
        dbg = {
            nm: nc.dram_tensor(f"dbg_{nm}", [128, N], FP16, kind="ExternalOutput")
            for nm in ("qT0", "qT1", "kT0", "kT1", "pT", "oT0", "oT1")
        }
        dbgP = nc.dram_tensor("dbg_P", [128, 2, N], FP16, kind="ExternalOutput")
        dbgpos = nc.dram_tensor("dbg_pos", [128, 2, N], FP16, kind="ExternalOutput")
        dbgav = nc.dram_tensor("dbg_av", [128, 2, D + 1], F32, kind="ExternalOutput")
        dbgosb = nc.dram_tensor("dbg_osb", [128, 2, D], FP16, kind="ExternalOutput")
        dbgptg = nc.dram_tensor("dbg_ptg", [128, 4, N], FP16, kind="ExternalOutput")
        dbgv = nc.dram_tensor("dbg_v", [128, NB, HPC, D + 1], FP16, kind="ExternalOutput")

    with tile.TileContext(nc) as tc:
        with contextlib.ExitStack() as ctx:
            const = ctx.enter_context(tc.tile_pool(name="const", bufs=1))
            pers = ctx.enter_context(tc.tile_pool(name="pers", bufs=1))

            # shared PSUM pools: every matmul in the kernel goes through these
            spp = ctx.enter_context(tc.tile_pool(name="spsum", bufs=SPP_BUFS, space="PSUM"))
            tpp = ctx.enter_context(tc.tile_pool(name="tpsum", bufs=TPP_BUFS, space="PSUM"))

            # ---- constants / persistent weights ---------------------------
            ident_sb = const.tile([128, 128], FP16, tag="ident")
            nc.scalar.dma_start(out=ident_sb, in_=ident[:, :])
            wo_sb = [pers.tile([128, DIM], FP16, tag=f"wo{p}", name=f"wo{p}")
                     for p in range(PAIRS)]
            for p in range(PAIRS):
                nc.scalar.dma_start(out=wo_sb[p], in_=wo[128 * p:128 * p + 128, :])

            # ---- persistent activations -----------------------------------
            qT = [pers.tile([128, N], FP16, tag=f"qT{p}", name=f"qT{p}") for p in range(PAIRS)]
            kT = [pers.tile([128, N], FP16, tag=f"kT{p}", name=f"kT{p}") for p in range(PAIRS)]
            pT = pers.tile([128, N], FP16, tag="pT")
            # v with a ones column per head: [j-part, jb, head, 65]
            v_sb = pers.tile([128, NB, HPC, D + 1], FP16, tag="v")
            oT = [pers.tile([128, N], FP16, tag=f"oT{p}", name=f"oT{p}") for p in range(PAIRS)]
            # transposed probs P^T, grouped 4 j-blocks per superblock (per pair)
            PTG = [
                [
                    pers.tile([128, 4, N - CH * Jg], FP16,
                              tag=f"PTG{half}_{Jg}", name=f"PTG{half}_{Jg}")
                    for Jg in range(NB // 4)
                ]
                for half in range(2)
            ]

            nc.vector.memset(v_sb[:, :, :, D:D + 1], 1.0)

            def emit_B_block(p, I, ubst):
                """U = q.p^T -> E = exp(U) -> shifted DRAM slab, one block."""
                i0 = 128 * I
                r0 = N - 128 - i0
                width = i0 + 128
                nchunks = -(-width // CH)
                ub2 = ubst.tile([128, 2, N], FP16, tag="ub2", name=f"ub2_{p}_{I}")
                for ci in range(nchunks):
                    rc = r0 + CH * ci
                    w = min(CH, N - rc)
                    psu = spp.tile([128, 2, CH], F32, tag="sp", name=f"psu_{p}_{I}_{ci}")
                    for half in range(2):
                        nc.tensor.matmul(
                            psu[:, half, :w],
                            qT[p][D * half:D * half + D, i0:i0 + 128],
                            pT[D * half:D * half + D, rc:rc + w],
                            start=True, stop=True,
                            tile_position=(D * half, 0),
                            skip_group_check=True,
                        )
                    oc = rc - r0
                    nc.scalar.activation(
                        out=ub2[:, :, oc:oc + w], in_=psu[:, :, :w],
                        func=mybir.ActivationFunctionType.Exp,
                    )
                dst = bass.AP(
                    tensor=EB[p],
                    offset=i0 * N + r0,
                    ap=[[N, 128], [N * N, 2], [1, width]],
                )
                nc.sync.dma_start(out=dst, in_=ub2[:, :, :width])

            def emit_qk_group(p, qk, g, x_t, wq_sb, wk_sb, bq_sb, bk_sb):
                """One [128, 1024] group of the q^T / k^T projection."""
                w_sb = wq_sb if qk == 0 else wk_sb
                dst = (qT if qk == 0 else kT)[p]
                sc = 1.0 if qk == 0 else SCALE
                bias = (bq_sb if qk == 0 else bk_sb)[:, p:p + 1]
                ps = spp.tile([128, 2, CH], F32, tag="sp", name=f"pqk{qk}_{p}_{g}")
                for kc in range(KC):
                    for h in range(2):
                        c = 2 * g + h
                        nc.tensor.matmul(
                            ps[:, h, :], w_sb[:, kc, 128 * p:128 * p + 128],
                            x_t[kc][:, CH * c:CH * c + CH],
                            start=(kc == 0), stop=(kc == KC - 1),
                            skip_group_check=True,
                        )
                nc.vector.tensor_scalar(
                    out=dst[:, CH * 2 * g:CH * 2 * g + 2 * CH], in0=ps,
                    scalar1=sc, scalar2=bias,
                    op0=ALU.mult, op1=ALU.add,
                )

            def emit_qk(p, x_t, wq_sb, wk_sb, bq_sb, bk_sb):
                for qk in range(2):
                    for g in range(2):
                        emit_qk_group(p, qk, g, x_t, wq_sb, wk_sb, bq_sb, bk_sb)

            d_state = {}

            def emit_D_half(I, ost):
                """half of an out-partial block: rows [128I, 128I+128);
                the write fires when the odd half completes."""
                Ip, b2 = I // 2, I % 2
                if b2 == 0:
                    d_state[Ip] = ost.tile([128, 2, DIM], FP16, tag="o2",
                                           name=f"o2_{Ip}", bufs=2)
                o2 = d_state[Ip]
                i0 = 128 * I
                pso = spp.tile([128, 2, CH], F32, tag="sp", name=f"pso_{I}")
                for c in range(2):
                    for p in range(PAIRS):
                        nc.tensor.matmul(
                            pso[:, c, :],
                            oT[p][:, i0:i0 + 128],
                            wo_sb[p][:, CH * c:CH * c + CH],
                            start=(p == 0), stop=(p == PAIRS - 1),
                            skip_group_check=True,
                        )
                if Ip >= 5 and b2 == 1:
                    # tail blocks: the exp stream has drained, ACT is free
                    nc.scalar.activation(
                        out=o2[:, b2, :], in_=pso,
                        func=mybir.ActivationFunctionType.Copy,
                    )
                else:
                    nc.vector.tensor_copy(out=o2[:, b2, :], in_=pso)
                if b2 == 1:
                    dst = bass.AP(
                        tensor=out,
                        offset=256 * Ip * DIM,
                        ap=[[DIM, 128], [128 * DIM, 2], [1, DIM]],
                    )
                    nc.sync.dma_start(out=dst, in_=o2)

            # staging for E blocks (outlives phase A: B(p1) interleaves with C(p0))
            ubst = ctx.enter_context(tc.tile_pool(name="ubstage", bufs=UB_BUFS))

            # ================= phase A + B(p0) =============================
            with contextlib.ExitStack() as s2:
                stream = s2.enter_context(tc.tile_pool(name="xstream", bufs=1))
                postream = s2.enter_context(tc.tile_pool(name="postream", bufs=3))

                # load order matters: wq then the x stream feed the very first
                # matmuls; everything else trails on the same queue
                wpd_sb = stream.tile([128, KC, 128], FP16, tag="wpd")
                nc.scalar.dma_start(out=wpd_sb, in_=wpd[:, :].rearrange("(kc p) m -> p kc m", p=128))
                bpd_sb = stream.tile([128, 1], F32, tag="bpd")
                nc.scalar.dma_start(out=bpd_sb, in_=bpd[:, :])
                bq_sb = stream.tile([128, PAIRS], F32, tag="bq")
                bk_sb = stream.tile([128, PAIRS], F32, tag="bk")
                for p in range(PAIRS):
                    nc.scalar.dma_start(out=bq_sb[:, p:p + 1], in_=bq[128 * p:128 * p + 128, :])
                    nc.scalar.dma_start(out=bk_sb[:, p:p + 1], in_=bks[128 * p:128 * p + 128, :])
                wq_sb = stream.tile([128, KC, 256], FP16, tag="wq")
                wk_sb = stream.tile([128, KC, 256], FP16, tag="wk")
                wv_sb = stream.tile([128, KC, 256], FP16, tag="wv")
                nc.sync.dma_start(out=wq_sb, in_=wq[:, :].rearrange("(kc p) m -> p kc m", p=128))
                x_t = []
                for kc in range(KC):
                    t = stream.tile([128, N], FP16, tag=f"xt{kc}")
                    nc.sync.dma_start(out=t, in_=xT[128 * kc:128 * kc + 128, :])
                    x_t.append(t)
                nc.sync.dma_start(out=wk_sb, in_=wk[:, :].rearrange("(kc p) m -> p kc m", p=128))
                nc.sync.dma_start(out=wv_sb, in_=wv[:, :].rearrange("(kc p) m -> p kc m", p=128))

                # posT loads go on the sync queue BEHIND the x tiles: the
                # shared DMA device then serves x first (q(p0) gates the
                # whole pipeline; p^T isn't needed until ~20us)
                pos_t = []
                for kc in range(KC):
                    t = postream.tile([128, N], FP16, tag="pos", name=f"post{kc}")
                    nc.sync.dma_start(out=t, in_=posT[128 * kc:128 * kc + 128, :])
                    pos_t.append(t)

                # only q(p0) before p^T: k(p0) is not needed until the
                # C(p0) scores, so it moves into the B(p0) filler slots and
                # the exp(U) stream starts ~25us earlier on the idle ACT;
                # qk(p1) runs before the p^T psum tiles are requested so the
                # long posT-wait doesn't starve the rotation
                for g in range(2):
                    emit_qk_group(0, 0, g, x_t, wq_sb, wk_sb, bq_sb, bk_sb)
                emit_qk(1, x_t, wq_sb, wk_sb, bq_sb, bk_sb)

                # p^T (dup-packed to 128 partitions); 4 chunks stay live
                # across the kc accumulation (2 spsum tiles)
                pts = [spp.tile([128, 2, CH], F32, tag="sp", name=f"ppt{t}")
                       for t in range(2)]
                for kc in range(KC):
                    for c in range(N // CH):
                        nc.tensor.matmul(
                            pts[c // 2][:, c % 2, :], wpd_sb[:, kc, :],
                            pos_t[kc][:, CH * c:CH * c + CH],
                            start=(kc == 0), stop=(kc == KC - 1),
                            skip_group_check=True,
                        )
                for c in range(N // CH):
                    nc.scalar.activation(
                        out=pT[:, CH * c:CH * c + CH], in_=pts[c // 2][:, c % 2, :],
                        func=mybir.ActivationFunctionType.Identity,
                        bias=bpd_sb, scale=SCALE,
                    )

                def emit_v_group(grp):
                    # v (no bias: folded to host), 2 j-blocks per psum tile:
                    # each accumulation group needs its own 2KB zero region
                    # (= psum bank), so only one group per bank half
                    psv = spp.tile([128, 2, CH], F32, tag="sp", name=f"psv{grp}")
                    for kc in range(KC):
                        for j in range(2):
                            jb = 2 * grp + j
                            nc.tensor.matmul(
                                psv[:, j, 0:256],
                                x_t[kc][:, 128 * jb:128 * jb + 128],
                                wv_sb[:, kc, :],
                                start=(kc == 0), stop=(kc == KC - 1),
                                skip_group_check=True,
                            )
                    for j in range(2):
                        jb = 2 * grp + j
                        nc.vector.tensor_copy(
                            out=v_sb[:, jb, :, 0:D],
                            in_=psv[:, j, 0:256],
                        )

                # B(p0) blocks interleaved with the qk(p1)/v projection
                # groups so PE work is always available while the activation
                # engine drains exp(U) from the shared psum pool
                for I in range(NB):
                    emit_B_block(0, I, ubst)
                    if I < 2:
                        emit_qk_group(0, 1, I, x_t, wq_sb, wk_sb, bq_sb, bk_sb)
                    elif I < 10:
                        emit_v_group(I - 2)

            # ================= phase C (+ D interleaved) ===================
            with contextlib.ExitStack() as s3:
                ptst = s3.enter_context(tc.tile_pool(name="ptstage", bufs=PT_BUFS))
                posst = s3.enter_context(tc.tile_pool(name="posstage", bufs=POS_BUFS))
                ost = s3.enter_context(tc.tile_pool(name="ostage", bufs=3))

                def c_front(p, I):
                    """qk matmuls + exp for block I; returns (pos2, Pt)."""
                    i0 = 128 * I
                    span = i0 + 128
                    nchunks = -(-span // CH)
                    pos2 = posst.tile([128, 2, N], FP16, tag="pos2", name=f"pos2_{p}_{I}")
                    src = bass.AP(
                        tensor=EB[p],
                        offset=i0 * (N - 1) + (N - 1),
                        ap=[[N - 1, 128], [N * N, 2], [1, span]],
                    )
                    nc.sync.dma_start(out=pos2[:, :, :span], in_=src)
                    Pt = ptst.tile([128, 2, N], FP16, tag="Pt", name=f"Pt_{p}_{I}")
                    for c in range(nchunks):
                        w = min(CH, span - CH * c)
                        psc = spp.tile([128, 2, CH], F32, tag="sp", name=f"psc_{p}_{I}_{c}")
                        for half in range(2):
                            nc.tensor.matmul(
                                psc[:, half, :w],
                                qT[p][D * half:D * half + D, i0:i0 + 128],
                                kT[p][D * half:D * half + D, CH * c:CH * c + w],
                                start=True, stop=True,
                                tile_position=(D * half, 0),
                                skip_group_check=True,
                            )
                        nc.scalar.activation(
                            out=Pt[:, :, CH * c:CH * c + w], in_=psc[:, :, :w],
                            func=mybir.ActivationFunctionType.Exp,
                        )
                    return pos2, Pt

                def c_mult(p, I, pos2, Pt):
                    """P = exp(qk) * E_shifted; causal diag mask on E first."""
                    i0 = 128 * I
                    span = i0 + 128
                    nchunks = -(-span // CH)
                    # keep s-i0 <= r in the diagonal 128 cols; masked and
                    # spill-garbage lanes are replaced by 0.0
                    nc.gpsimd.affine_select(
                        out=pos2[:, :, i0:i0 + 128],
                        in_=pos2[:, :, i0:i0 + 128],
                        pattern=[[0, 2], [-1, 128]],
                        compare_op=ALU.is_ge,
                        fill=0.0,
                        channel_multiplier=1,
                    )
                    for c in range(nchunks):
                        w = min(CH, span - CH * c)
                        pool_mult = (c % 2 == 1) if I < 11 else (c % 3 != 0)
                        eng = nc.gpsimd if pool_mult else nc.vector
                        eng.tensor_tensor(
                            out=Pt[:, :, CH * c:CH * c + w],
                            in0=Pt[:, :, CH * c:CH * c + w],
                            in1=pos2[:, :, CH * c:CH * c + w],
                            op=ALU.mult,
                        )

                def c_back_a(p, I, Pt):
                    """transposes, PTG copies, attn@v."""
                    i0 = 128 * I
                    for half in range(2):
                        for Jg in range((I + 4) // 4):
                            nj = min(4, I + 1 - 4 * Jg)
                            pstw = tpp.tile([128, 4, 128], FP16, tag="pst")
                            for t in range(nj):
                                J = 4 * Jg + t
                                nc.tensor.transpose(
                                    pstw[:, t, :], Pt[:, half, 128 * J:128 * J + 128],
                                    ident_sb,
                                )
                            off = i0 - CH * Jg
                            # GPSIMD cannot read PSUM on this hw: psum->sbuf
                            # copies stay on DVE
                            nc.vector.tensor_copy(
                                out=PTG[half][Jg][:, :nj, off:off + 128],
                                in_=pstw[:, :nj, :],
                            )

                    # attn @ v (flipped layout) + ones-column denominator
                    psav = spp.tile([128, 2, CH], F32, tag="sp", name=f"psav_{p}_{I}")
                    for half in range(2):
                        h = 2 * p + half
                        for J in range(I + 1):
                            ioff = i0 - CH * (J // 4)
                            nc.tensor.matmul(
                                psav[:, half, :D + 1],
                                PTG[half][J // 4][:, J % 4, ioff:ioff + 128],
                                v_sb[:, J, h, :],
                                start=(J == 0), stop=(J == I),
                                skip_group_check=True,
                            )
                    return psav

                def c_back_b(p, I, psav):
                    """normalize: o = av / den, then transpose into oT."""
                    i0 = 128 * I
                    if debug and p == 0 and I == 4:
                        avs = ost.tile([128, 2, D + 1], F32, tag="dbgav")
                        nc.vector.tensor_copy(out=avs, in_=psav[:, :, :D + 1])
                        nc.sync.dma_start(out=dbgav[:, :, :], in_=avs)
                    rcp = ost.tile([128, 2, 1], F32, tag="rcp", name=f"rcp_{p}_{I}")
                    nc.vector.reciprocal(out=rcp, in_=psav[:, :, D:D + 1])
                    o_sb = ost.tile([128, 2, D], FP16, tag="osb", name=f"osb_{p}_{I}")
                    for half in range(2):
                        nc.vector.tensor_scalar(
                            out=o_sb[:, half, :], in0=psav[:, half, 0:D],
                            scalar1=rcp[:, half, :], scalar2=None,
                            op0=ALU.mult,
                        )
                    if debug and p == 0 and I == 4:
                        nc.sync.dma_start(out=dbgosb[:, :, :], in_=o_sb)
                    pso2 = tpp.tile([128, 4, 128], FP16, tag="pst", name=f"psot_{p}_{I}")
                    nc.tensor.transpose(
                        pso2[:, 0, :], o_sb[:, :, :], ident_sb,
                    )
                    nc.vector.tensor_copy(
                        out=oT[p][:, i0:i0 + 128], in_=pso2[:, 0, :],
                    )

                # one-deep software pipeline: while block I's exp runs on the
                # activation engine, the PE works on block I-1's transposes
                # and attn@v instead of stalling; DVE interleaves I's P*E
                # multiply between I-1's PTG copies and normalize.  B(p1)
                # blocks interleave with the C(p0) pipeline so the activation
                # engine alternates between exp(U) and exp(qk) streams, and
                # the out-projection blocks interleave into the C(p1) loop.
                # two-block pipeline stages: each engine gets ~2 blocks of
                # work per cross-engine handoff, amortizing semaphore latency
                cstate = {p: {"prev2": [], "d_done": set()} for p in range(PAIRS)}

                def c_stage(p, stage):
                    st = cstate[p]
                    prev2 = st["prev2"]
                    cur2 = [(I,) + c_front(p, I) for I in stage]
                    psavs = [c_back_a(p, pv[0], pv[2]) for pv in prev2]
                    for cu in cur2:
                        c_mult(p, cu[0], cu[1], cu[2])
                    for pv, psav in zip(prev2, psavs):
                        c_back_b(p, pv[0], psav)
                    if debug:
                        for cu in cur2:
                            if p == 0 and cu[0] == 4:
                                nc.sync.dma_start(out=dbgpos[:, :, :], in_=cu[1])
                                nc.sync.dma_start(out=dbgP[:, :, :], in_=cu[2])
                    if prev2 and p == 1:
                        done = {pv[0] for pv in prev2}
                        d_done = st["d_done"]
                        for pI in sorted(done):
                            # a pair's out-projection fires once both of
                            # its row blocks are normalized
                            if pI % 2 == 0 and (pI + 1 in done or pI + 1 in d_done):
                                emit_D_half(pI, ost)
                                emit_D_half(pI + 1, ost)
                            elif pI % 2 == 1 and pI - 1 in d_done:
                                emit_D_half(pI - 1, ost)
                                emit_D_half(pI, ost)
                        d_done.update(done)
                    st["prev2"] = cur2

                def stage_list():
                    stages = [[s, s + 1] for s in range(0, NB - 2, 2)]
                    stages += [[NB - 2], [NB - 1], []]
                    return stages

                # merged schedule: C(p1)'s first stages overlap C(p0)'s
                # final (smallest-work) stages -- their PTG column ranges
                # are disjoint from anything C(p0) still reads, so the
                # transition dip between the pair loops is filled
                LAP = 4  # p1 stages running during p0 tail
                st0, st1 = stage_list(), stage_list()
                for si, stage in enumerate(st0):
                    if si < 8:
                        emit_B_block(1, si, ubst)
                    c_stage(0, stage)
                    sj = si - (len(st0) - 1 - LAP)
                    if 0 <= sj < LAP:
                        if sj < 8:
                            emit_B_block(1, 8 + sj, ubst)
                        c_stage(1, st1[sj])
                for sj in range(LAP, len(st1)):
                    if sj < 8:
                        emit_B_block(1, 8 + sj, ubst)
                    c_stage(1, st1[sj])

                if debug:
                    nc.sync.dma_start(out=dbgptg[:, :, :], in_=PTG[0][0])
                if debug:
                    for nm, t in (("qT0", qT[0]), ("qT1", qT[1]), ("kT0", kT[0]),
                                  ("kT1", kT[1]), ("pT", pT), ("oT0", oT[0]),
                                  ("oT1", oT[1])):
                        nc.sync.dma_start(out=dbg[nm][:, :], in_=t)
                    nc.sync.dma_start(out=dbgv[:, :, :, :], in_=v_sb)

    _patch_bass(nc)
    return nc


_NC_CACHE = {}


def _get_nc():
    if "nc" not in _NC_CACHE:
        _NC_CACHE["nc"] = build_nc()
    return _NC_CACHE["nc"]


def kernel(x, pos_emb, Wq, bq, Wkv, bkv, Wp, bp, Wo, bo):
    x = np.asarray(x, dtype=np.float32)
    pos_emb = np.asarray(pos_emb, dtype=np.float32)
    Wq = np.asarray(Wq, dtype=np.float32)
    bq = np.asarray(bq, dtype=np.float32)
    Wkv = np.asarray(Wkv, dtype=np.float32)
    bkv = np.asarray(bkv, dtype=np.float32)
    Wp = np.asarray(Wp, dtype=np.float32)
    bp = np.asarray(bp, dtype=np.float32)
    Wo = np.asarray(Wo, dtype=np.float32)
    bo = np.asarray(bo, dtype=np.float32)

    b, n, dim = x.shape
    assert (b, n, dim) == (2, N, DIM)

    xTs = [np.ascontiguousarray(x[bi].T).astype(np.float16) for bi in range(b)]
    posT = np.ascontiguousarray(pos_emb.T).astype(np.float16)
    wpd = np.concatenate([Wp, Wp], axis=1).astype(np.float16)
    bpd = np.concatenate([bp, bp])[:, None].astype(np.float32) * SCALE
    ident = np.eye(128, dtype=np.float16)

    in_maps = []
    for c in range(8):
        bi, g = divmod(c, HPC)
        cols = slice(256 * g, 256 * g + 256)
        in_maps.append(
            {
                "xT": xTs[bi],
                "posT": posT,
                "wq": np.ascontiguousarray(Wq[:, cols]).astype(np.float16),
                "wk": np.ascontiguousarray(Wkv[:, 256 * g:256 * g + 256]).astype(np.float16),
                "wv": np.ascontiguousarray(
                    Wkv[:, DIM + 256 * g:DIM + 256 * g + 256]).astype(np.float16),
                "wpd": wpd,
                "wo": np.ascontiguousarray(Wo[256 * g:256 * g + 256, :]).astype(np.float16),
                "bq": np.ascontiguousarray(bq[cols])[:, None].astype(np.float32),
                "bks": (np.ascontiguousarray(bkv[256 * g:256 * g + 256]) * SCALE)[:, None]
                .astype(np.float32),
                "bpd": bpd,
                "ident": ident,
            }
        )

    nc = _get_nc()
    res = run_bass_kernel_spmd(nc, in_maps, core_ids=list(range(8)))

    outp = np.zeros((b, n, dim), dtype=np.float32)
    for c in range(8):
        bi = c // HPC
        outp[bi] += res.results[c]["out"].astype(np.float32)
    # v-bias folds through softmax (rows sum to 1): out += b_v @ Wo
    bv = bkv[DIM:]
    outp += bo + (bv.astype(np.float64) @ Wo.astype(np.float64)).astype(np.float32)
    return outp.astype(np.float32)
